# revision 24
# baseline (speedup 1.0000x reference)
"""Multi-head causal attention (B=2,S=2048,E=1024,H=16,D=64) on 8 NeuronCores.

Sharding: core c handles batch b=c//4 and head-group hg=c%4 (4 heads = 256
channels each).  Each core computes Q^T/K^T/V projections for its channel
slice, causal softmax attention for its 4 heads, and a partial output
projection through its slice of Wo.  Host sums the 4 partials per batch and
adds the bias.

All matmuls run as float32r (TF32-like: full-rate, ~1.6e-4 rel err).
Attention dataflow is transpose-free:
  scores^T[k,q] = (K^T chunk)^T-stationary @ Q^T      (d on partitions)
  P^T = exp(scores^T * D^-0.5)                        (ACT, causal-restricted)
  out^T[d,q] (+ denom row) = [V|1]^T-stationary @ P^T (k on partitions)
  normalize by broadcasted 1/denom, then
  partial[s,e] = (out_norm^T chunk)-stationary @ Wo-slice
"""

import sys

sys.path.insert(0, "/opt/trn_rl_repo")

import numpy as np

B, S, E, H, D = 2, 2048, 1024, 16, 64
N_CORES = 8
HPC = 4               # heads per core
CH = HPC * D          # 256 channels per core
SBK = 512             # seq block (moving free dim)
NSB = S // SBK        # 4
NE = E // 128         # 8 contraction chunks
NKC = S // 128        # 16 key chunks

_BUILT = {}


def _build():
    if "nc" in _BUILT:
        return _BUILT["nc"]

    from contextlib import ExitStack

    import concourse.bacc as bacc
    import concourse.tile as tile
    from concourse import mybir

    F32 = mybir.dt.float32
    F32R = mybir.dt.float32r
    BF16 = mybir.dt.bfloat16
    AF = mybir.ActivationFunctionType

    nc = bacc.Bacc("TRN2", target_bir_lowering=False, debug=False,
                   num_devices=N_CORES)
    xt = nc.dram_tensor("xt", [E, S], BF16, kind="ExternalInput").ap()
    wq = nc.dram_tensor("wq", [E, CH], BF16, kind="ExternalInput").ap()
    wk = nc.dram_tensor("wk", [E, CH], BF16, kind="ExternalInput").ap()
    wv = nc.dram_tensor("wv", [E, CH], BF16, kind="ExternalInput").ap()
    wo = nc.dram_tensor("wo", [CH, E], BF16, kind="ExternalInput").ap()
    tri = nc.dram_tensor("tri", [128, 128], BF16, kind="ExternalInput").ap()
    pout = nc.dram_tensor("pout", [S, E], F32, kind="ExternalOutput").ap()

    with tile.TileContext(nc) as tc, ExitStack() as ctx:
        wop = ctx.enter_context(tc.tile_pool(name="wop", bufs=2))
        qkp = ctx.enter_context(tc.tile_pool(name="qkp", bufs=4))
        vp = ctx.enter_context(tc.tile_pool(name="vp", bufs=NKC))
        trip = ctx.enter_context(tc.tile_pool(name="trip", bufs=1))
        pp = ctx.enter_context(tc.tile_pool(name="pp", bufs=2, space="PSUM"))
        sp = ctx.enter_context(tc.tile_pool(name="sp", bufs=4, space="PSUM"))
        avp = ctx.enter_context(tc.tile_pool(name="avp", bufs=2, space="PSUM"))
        xtp = ctx.enter_context(tc.tile_pool(name="xtp", bufs=NE))
        wp = ctx.enter_context(tc.tile_pool(name="wp", bufs=3 * NE))

        # --- loads: tri (warmup dep) + x^T first (critical path) ---
        tri_sb = trip.tile([128, 128], BF16, tag="tri")
        nc.sync.dma_start(tri_sb[:], tri[:, :])
        ones_sb = trip.tile([128, HPC], BF16, tag="ones")
        nc.vector.memset(ones_sb[:], 1.0)
        xts = []
        for e in range(NE):
            t = xtp.tile([128, S], BF16, tag="xt")
            nc.sync.dma_start(t[:], xt[e * 128:(e + 1) * 128, :])
            xts.append(t)
        wqs, wks, wvs = [], [], []
        for lst, srcw, tg in ((wqs, wq, "wq"), (wks, wk, "wk"),
                              (wvs, wv, "wv")):
            for e in range(NE):
                t = wp.tile([128, CH], BF16, tag=tg)
                nc.gpsimd.dma_start(t[:], srcw[e * 128:(e + 1) * 128, :])
                lst.append(t)
        wos = []
        for cc in range(2):
            t = wop.tile([128, E], BF16, tag="wo")
            nc.gpsimd.dma_start(t[:], wo[cc * 128:(cc + 1) * 128, :])
            wos.append(t)

        onp = ctx.enter_context(tc.tile_pool(name="onp", bufs=2))
        ptp = ctx.enter_context(tc.tile_pool(name="ptp", bufs=8))
        recp = ctx.enter_context(tc.tile_pool(name="recp", bufs=2))
        bcp = ctx.enter_context(tc.tile_pool(name="bcp", bufs=2))
        oop = ctx.enter_context(tc.tile_pool(name="oop", bufs=3))
        ons = [onp.tile([128, S], BF16, tag="on", name=f"on{i}")
               for i in range(2)]

        qkt = {"q": [qkp.tile([128, S], BF16, tag="qk", name=f"qt{i}")
                     for i in range(2)],
               "k": [qkp.tile([128, S], BF16, tag="qk", name=f"kt{i}")
                     for i in range(2)]}
        vts = [vp.tile([128, HPC * 65], BF16, tag="v", name=f"v{i}")
               for i in range(NKC)]

        # ---- dense-matmul group emitters (HAM-warming filler work) ----
        def qk_group(name, wts, cc, sb, sprinkle=0):
            ps = pp.tile([128, SBK], F32, tag="pp", name=f"ps_{name}{cc}{sb}")
            for e in range(NE):
                nc.tensor.matmul(
                    ps[:], lhsT=wts[e][:, cc * 128:(cc + 1) * 128],
                    rhs=xts[e][:, sb * SBK:(sb + 1) * SBK],
                    start=(e == 0), stop=(e == NE - 1))
                if sprinkle:
                    warm_mm(sprinkle)
            nc.scalar.copy(qkt[name][cc][:, sb * SBK:(sb + 1) * SBK], ps[:])

        def v_group(sc, sprinkle=0):
            ps = pp.tile([128, CH], F32, tag="pp", name=f"ps_v{sc}")
            for e in range(NE):
                nc.tensor.matmul(ps[:], lhsT=xts[e][:, sc * 128:(sc + 1) * 128],
                                 rhs=wvs[e][:], start=(e == 0),
                                 stop=(e == NE - 1))
                if sprinkle:
                    warm_mm(sprinkle)
            dst = vts[sc][:].rearrange("p (h c) -> p h c", h=HPC)
            nc.vector.tensor_copy(dst[:, :, 0:64],
                                  ps[:].rearrange("p (h c) -> p h c", h=HPC))
            nc.vector.tensor_copy(dst[:, :, 64:65], ones_sb[:].unsqueeze(2))

        woh = {}

        def wo_half(sc, eb):
            ps = pp.tile([128, SBK], F32, tag="pp", name=f"ph_o{sc}{eb}")
            nc.tensor.matmul(ps[:], lhsT=ons[0][:, sc * 128:(sc + 1) * 128],
                             rhs=wos[0][:, eb * SBK:(eb + 1) * SBK],
                             start=True, stop=True)
            t = oop.tile([128, SBK], F32, tag="oh", bufs=8,
                         name=f"oh{sc}{eb}")
            nc.vector.tensor_copy(t[:], ps[:])
            woh[(sc, eb)] = t

        def wo_combine(sc, eb):
            ps = pp.tile([128, SBK], F32, tag="pp", name=f"pc_o{sc}{eb}")
            nc.tensor.matmul(ps[:], lhsT=ons[1][:, sc * 128:(sc + 1) * 128],
                             rhs=wos[1][:, eb * SBK:(eb + 1) * SBK],
                             start=True, stop=True)
            oo = oop.tile([128, SBK], F32, tag="oo", name=f"oc{sc}{eb}")
            nc.vector.tensor_add(oo[:], ps[:], woh[(sc, eb)][:])
            nc.sync.dma_start(
                pout[sc * 128:(sc + 1) * 128, eb * SBK:(eb + 1) * SBK],
                oo[:])

        def wo_group(sc, eb):
            ps = pp.tile([128, SBK], F32, tag="pp", name=f"ps_o{sc}{eb}")
            for cc in range(2):
                nc.tensor.matmul(ps[:],
                                 lhsT=ons[cc][:, sc * 128:(sc + 1) * 128],
                                 rhs=wos[cc][:, eb * SBK:(eb + 1) * SBK],
                                 start=(cc == 0), stop=(cc == 1))
            oo = oop.tile([128, SBK], F32, tag="oo", name=f"oo{sc}{eb}")
            nc.vector.tensor_copy(oo[:], ps[:])
            nc.sync.dma_start(
                pout[sc * 128:(sc + 1) * 128, eb * SBK:(eb + 1) * SBK],
                oo[:])

        def warm_mm(n):
            for _ in range(n):
                wps = sp.tile([128, 128], F32, tag="sp", name="warmps")
                nc.tensor.matmul(wps[:], lhsT=tri_sb[:], rhs=tri_sb[:],
                                 start=True, stop=True)

        from collections import deque
        fillers = deque()
        warm_mm(40)

        # prologue: everything attention (qb0, h0/h1) needs
        for sb in range(NSB):
            qk_group("q", wqs, 0, sb, sprinkle=1)
        for sb in range(NSB):
            qk_group("k", wks, 0, sb, sprinkle=1)
        for sc in range(4):
            v_group(sc, sprinkle=1)
        # filler consumed during qb0: projections for heads 2-3, V for qb1
        for sb in range(NSB):
            fillers.append(lambda sb=sb: qk_group("q", wqs, 1, sb))
        for sb in range(NSB):
            fillers.append(lambda sb=sb: qk_group("k", wks, 1, sb))
        for sc in range(4, 8):
            fillers.append(lambda sc=sc: v_group(sc))

        # ---- attention: qb-outer so Wo/output-DMA spread across the run ----
        for qb in range(NSB):
            if qb >= 1:
                for sc in range(4 * (qb + 1), 4 * (qb + 2)):
                    if sc < NKC:
                        fillers.append(lambda sc=sc: v_group(sc))
                for sc in range(4 * (qb - 1), 4 * qb):
                    for eb in range(2):
                        fillers.append(
                            lambda sc=sc, eb=eb: wo_group(sc, eb))
            for h in range(HPC):
                if qb == NSB - 1 and h == 2:
                    # h0/h1 of the last q-block are done: their half of the
                    # final output projection can overlap h2/h3
                    for sc in range(12, NKC):
                        for eb in range(2):
                            fillers.append(
                                lambda sc=sc, eb=eb: wo_half(sc, eb))
                cc, po = h // 2, (h % 2) * 64
                qt, kt = qkt["q"][cc], qkt["k"][cc]
                av = avp.tile([65, SBK], F32, tag="av", name=f"av{qb}{h}")
                nk = 4 * (qb + 1)
                pend = deque()  # AV lags scores; emit in same-shape pairs
                def flush_av(nmax, final=False):
                    n = 0
                    while pend and n < nmax:
                        pkc, pj0, ppt = pend.popleft()
                        nc.tensor.matmul(
                            av[:, pj0:SBK],
                            lhsT=vts[pkc][:, h * 65:(h + 1) * 65],
                            rhs=ppt[:, pj0:SBK],
                            start=(pkc == 0),
                            stop=(final and not pend),
                            skip_group_check=True)
                        n += 1
                for kc2 in range(0, nk, 2):
                    for kc in (kc2, kc2 + 1):
                        if kc >= nk:
                            break
                        k0 = kc * 128
                        j0 = max(0, k0 - qb * SBK)
                        ss = sp.tile([128, SBK], F32, tag="sp")
                        nc.tensor.matmul(
                            ss[:, j0:SBK],
                            lhsT=kt[po:po + 64, k0:k0 + 128],
                            rhs=qt[po:po + 64, qb * SBK + j0:(qb + 1) * SBK],
                            start=True, stop=True)
                        pt = ptp.tile([128, SBK], BF16, tag="pt")
                        nc.scalar.activation(pt[:, j0:SBK], ss[:, j0:SBK],
                                             AF.Exp, scale=float(D) ** -0.5)
                        if k0 >= qb * SBK:  # diag chunk: mask 128-wide band
                            nc.vector.tensor_mul(pt[:, j0:j0 + 128],
                                                 pt[:, j0:j0 + 128], tri_sb[:])
                        pend.append((kc, j0, pt))
                        if len(pend) > 2:
                            flush_av(1)
                        if fillers:
                            fillers.popleft()()
                flush_av(99, final=True)
                rec = recp.tile([1, SBK], F32, tag="rec")
                nc.vector.tensor_copy(rec[:], av[64:65, :])
                rec2 = recp.tile([1, SBK], F32, tag="rec2")
                nc.vector.reciprocal_approx_fast(rec2[:], rec[:])
                bc = bcp.tile([64, SBK], F32, tag="bc")
                nc.gpsimd.partition_broadcast(bc[:], rec2[:])
                nc.vector.tensor_mul(
                    ons[cc][po:po + 64, qb * SBK:(qb + 1) * SBK],
                    av[0:64, :], bc[:])

        while fillers:
            fillers.popleft()()
        # tail: combine the remaining half of the last q-block's projection
        for sc in range(12, NKC):
            for eb in range(2):
                wo_combine(sc, eb)


    nc.compile()
    _BUILT["nc"] = nc
    return nc


def _install_ntff_shim():
    """Provide antenv.axon_hooks (missing in this image) so trace=True works."""
    import types
    try:
        from antenv.axon_hooks import get_axon_ntff_profile_hook  # noqa: F401
        return
    except ImportError:
        pass
    import antenv
    from trn_agent_boot.trn_boot import _ntff_profile_via_ctypes
    hook = _ntff_profile_via_ctypes("/opt/axon/libaxon_pjrt.so")
    mod = types.ModuleType("antenv.axon_hooks")
    mod._hook = hook
    mod.get_axon_ntff_profile_hook = lambda: mod._hook
    mod.set_axon_ntff_profile_hook = lambda h: setattr(mod, "_hook", h)
    sys.modules["antenv.axon_hooks"] = mod
    antenv.axon_hooks = mod


def kernel(x, Wq, Wk, Wv, Wo, bo, _trace=False):
    from concourse.bass_utils import run_bass_kernel_spmd

    nc = _build()

    x = np.asarray(x, dtype=np.float32)
    Wq = np.asarray(Wq, dtype=np.float32)
    Wk = np.asarray(Wk, dtype=np.float32)
    Wv = np.asarray(Wv, dtype=np.float32)
    Wo = np.asarray(Wo, dtype=np.float32)
    bo = np.asarray(bo, dtype=np.float32)

    import ml_dtypes
    bf = ml_dtypes.bfloat16
    tri = np.triu(np.ones((128, 128), dtype=np.float32)).astype(bf)
    xt_b = [np.ascontiguousarray(x[b].T) for b in range(B)]
    in_maps = []
    for c in range(N_CORES):
        b, hg = c // HPC, c % HPC
        sl = slice(hg * CH, (hg + 1) * CH)
        in_maps.append({
            "xt": xt_b[b].astype(bf),
            "wq": np.ascontiguousarray(Wq[:, sl]).astype(bf),
            "wk": np.ascontiguousarray(Wk[:, sl]).astype(bf),
            "wv": np.ascontiguousarray(Wv[:, sl]).astype(bf),
            "wo": np.ascontiguousarray(Wo[sl, :]).astype(bf),
            "tri": tri,
        })

    kwargs = {}
    if _trace:
        _install_ntff_shim()
        kwargs = dict(trace=True, trace_cores=[0])
    res = run_bass_kernel_spmd(nc, in_maps, core_ids=list(range(N_CORES)),
                               **kwargs)

    out = np.zeros((B, S, E), dtype=np.float32)
    for c in range(N_CORES):
        out[c // HPC] += res.results[c]["pout"]
    out += bo
    if _trace:
        return out, res
    return out


# revision 26
# speedup vs baseline: 1.0022x; 1.0022x over previous
"""Multi-head causal attention (B=2,S=2048,E=1024,H=16,D=64) on 8 NeuronCores.

Sharding: core c handles batch b=c//4 and head-group hg=c%4 (4 heads = 256
channels each).  Each core computes Q^T/K^T/V projections for its channel
slice, causal softmax attention for its 4 heads, and a partial output
projection through its slice of Wo.  Host sums the 4 partials per batch and
adds the bias.

All matmuls run as float32r (TF32-like: full-rate, ~1.6e-4 rel err).
Attention dataflow is transpose-free:
  scores^T[k,q] = (K^T chunk)^T-stationary @ Q^T      (d on partitions)
  P^T = exp(scores^T * D^-0.5)                        (ACT, causal-restricted)
  out^T[d,q] (+ denom row) = [V|1]^T-stationary @ P^T (k on partitions)
  normalize by broadcasted 1/denom, then
  partial[s,e] = (out_norm^T chunk)-stationary @ Wo-slice
"""

import sys

sys.path.insert(0, "/opt/trn_rl_repo")

import numpy as np

B, S, E, H, D = 2, 2048, 1024, 16, 64
N_CORES = 8
HPC = 4               # heads per core
CH = HPC * D          # 256 channels per core
SBK = 512             # seq block (moving free dim)
NSB = S // SBK        # 4
NE = E // 128         # 8 contraction chunks
NKC = S // 128        # 16 key chunks

_BUILT = {}


def _build():
    if "nc" in _BUILT:
        return _BUILT["nc"]

    import concourse.bass_utils as _bu
    if not getattr(_bu, "_ldw_patched", False):
        _orig_run = _bu.run_command

        def _patched(cmd, *a, **kw):
            cmd = [c.replace("--enable-ldw-opt=false", "--enable-ldw-opt=false")
                   if isinstance(c, str) else c for c in cmd]
            return _orig_run(cmd, *a, **kw)

        _bu.run_command = _patched
        _bu._ldw_patched = True

    from contextlib import ExitStack

    import concourse.bacc as bacc
    import concourse.tile as tile
    from concourse import mybir

    F32 = mybir.dt.float32
    F32R = mybir.dt.float32r
    BF16 = mybir.dt.bfloat16
    AF = mybir.ActivationFunctionType

    nc = bacc.Bacc("TRN2", target_bir_lowering=False, debug=False,
                   num_devices=N_CORES)
    xt = nc.dram_tensor("xt", [E, S], BF16, kind="ExternalInput").ap()
    wq = nc.dram_tensor("wq", [E, CH], BF16, kind="ExternalInput").ap()
    wk = nc.dram_tensor("wk", [E, CH], BF16, kind="ExternalInput").ap()
    wv = nc.dram_tensor("wv", [E, CH], BF16, kind="ExternalInput").ap()
    wo = nc.dram_tensor("wo", [CH, E], BF16, kind="ExternalInput").ap()
    tri = nc.dram_tensor("tri", [128, 128], BF16, kind="ExternalInput").ap()
    pout = nc.dram_tensor("pout", [S, E], F32, kind="ExternalOutput").ap()

    with tile.TileContext(nc) as tc, ExitStack() as ctx:
        wop = ctx.enter_context(tc.tile_pool(name="wop", bufs=2))
        qkp = ctx.enter_context(tc.tile_pool(name="qkp", bufs=4))
        vp = ctx.enter_context(tc.tile_pool(name="vp", bufs=NKC))
        trip = ctx.enter_context(tc.tile_pool(name="trip", bufs=1))
        pp = ctx.enter_context(tc.tile_pool(name="pp", bufs=2, space="PSUM"))
        sp = ctx.enter_context(tc.tile_pool(name="sp", bufs=4, space="PSUM"))
        avp = ctx.enter_context(tc.tile_pool(name="avp", bufs=2, space="PSUM"))
        xtp = ctx.enter_context(tc.tile_pool(name="xtp", bufs=NE))
        wp = ctx.enter_context(tc.tile_pool(name="wp", bufs=3 * NE))

        # --- loads: tri (warmup dep) + x^T first (critical path) ---
        tri_sb = trip.tile([128, 128], BF16, tag="tri")
        nc.sync.dma_start(tri_sb[:], tri[:, :])
        ones_sb = trip.tile([128, HPC], BF16, tag="ones")
        nc.vector.memset(ones_sb[:], 1.0)
        xts = []
        for e in range(NE):
            t = xtp.tile([128, S], BF16, tag="xt")
            nc.sync.dma_start(t[:], xt[e * 128:(e + 1) * 128, :])
            xts.append(t)
        wqs, wks, wvs = [], [], []
        for lst, srcw, tg in ((wqs, wq, "wq"), (wks, wk, "wk"),
                              (wvs, wv, "wv")):
            for e in range(NE):
                t = wp.tile([128, CH], BF16, tag=tg)
                nc.gpsimd.dma_start(t[:], srcw[e * 128:(e + 1) * 128, :])
                lst.append(t)
        wos = []
        for cc in range(2):
            t = wop.tile([128, E], BF16, tag="wo")
            nc.gpsimd.dma_start(t[:], wo[cc * 128:(cc + 1) * 128, :])
            wos.append(t)

        onp = ctx.enter_context(tc.tile_pool(name="onp", bufs=2))
        ptp = ctx.enter_context(tc.tile_pool(name="ptp", bufs=8))
        recp = ctx.enter_context(tc.tile_pool(name="recp", bufs=2))
        bcp = ctx.enter_context(tc.tile_pool(name="bcp", bufs=2))
        oop = ctx.enter_context(tc.tile_pool(name="oop", bufs=3))
        ons = [onp.tile([128, S], BF16, tag="on", name=f"on{i}")
               for i in range(2)]

        qkt = {"q": [qkp.tile([128, S], BF16, tag="qk", name=f"qt{i}")
                     for i in range(2)],
               "k": [qkp.tile([128, S], BF16, tag="qk", name=f"kt{i}")
                     for i in range(2)]}
        vts = [vp.tile([128, HPC * 65], BF16, tag="v", name=f"v{i}")
               for i in range(NKC)]

        # ---- dense-matmul group emitters (HAM-warming filler work) ----
        def qk_group(name, wts, cc, sb, sprinkle=0):
            ps = pp.tile([128, SBK], F32, tag="pp", name=f"ps_{name}{cc}{sb}")
            for e in range(NE):
                nc.tensor.matmul(
                    ps[:], lhsT=wts[e][:, cc * 128:(cc + 1) * 128],
                    rhs=xts[e][:, sb * SBK:(sb + 1) * SBK],
                    start=(e == 0), stop=(e == NE - 1))
                if sprinkle:
                    warm_mm(sprinkle)
            nc.scalar.copy(qkt[name][cc][:, sb * SBK:(sb + 1) * SBK], ps[:])

        def v_group(sc, sprinkle=0):
            ps = pp.tile([128, CH], F32, tag="pp", name=f"ps_v{sc}")
            for e in range(NE):
                nc.tensor.matmul(ps[:], lhsT=xts[e][:, sc * 128:(sc + 1) * 128],
                                 rhs=wvs[e][:], start=(e == 0),
                                 stop=(e == NE - 1))
                if sprinkle:
                    warm_mm(sprinkle)
            dst = vts[sc][:].rearrange("p (h c) -> p h c", h=HPC)
            nc.vector.tensor_copy(dst[:, :, 0:64],
                                  ps[:].rearrange("p (h c) -> p h c", h=HPC))
            nc.vector.tensor_copy(dst[:, :, 64:65], ones_sb[:].unsqueeze(2))

        woh = {}

        def wo_half(sc, eb):
            ps = pp.tile([128, SBK], F32, tag="pp", name=f"ph_o{sc}{eb}")
            nc.tensor.matmul(ps[:], lhsT=ons[0][:, sc * 128:(sc + 1) * 128],
                             rhs=wos[0][:, eb * SBK:(eb + 1) * SBK],
                             start=True, stop=True)
            t = oop.tile([128, SBK], F32, tag="oh", bufs=8,
                         name=f"oh{sc}{eb}")
            nc.vector.tensor_copy(t[:], ps[:])
            woh[(sc, eb)] = t

        def wo_combine(sc, eb):
            ps = pp.tile([128, SBK], F32, tag="pp", name=f"pc_o{sc}{eb}")
            nc.tensor.matmul(ps[:], lhsT=ons[1][:, sc * 128:(sc + 1) * 128],
                             rhs=wos[1][:, eb * SBK:(eb + 1) * SBK],
                             start=True, stop=True)
            oo = oop.tile([128, SBK], F32, tag="oo", name=f"oc{sc}{eb}")
            nc.vector.tensor_add(oo[:], ps[:], woh[(sc, eb)][:])
            nc.sync.dma_start(
                pout[sc * 128:(sc + 1) * 128, eb * SBK:(eb + 1) * SBK],
                oo[:])

        def wo_group(sc, eb):
            ps = pp.tile([128, SBK], F32, tag="pp", name=f"ps_o{sc}{eb}")
            for cc in range(2):
                nc.tensor.matmul(ps[:],
                                 lhsT=ons[cc][:, sc * 128:(sc + 1) * 128],
                                 rhs=wos[cc][:, eb * SBK:(eb + 1) * SBK],
                                 start=(cc == 0), stop=(cc == 1))
            oo = oop.tile([128, SBK], F32, tag="oo", name=f"oo{sc}{eb}")
            nc.vector.tensor_copy(oo[:], ps[:])
            nc.sync.dma_start(
                pout[sc * 128:(sc + 1) * 128, eb * SBK:(eb + 1) * SBK],
                oo[:])

        def warm_mm(n):
            for _ in range(n):
                wps = sp.tile([128, 128], F32, tag="sp", name="warmps")
                nc.tensor.matmul(wps[:], lhsT=tri_sb[:], rhs=tri_sb[:],
                                 start=True, stop=True)

        from collections import deque
        fillers = deque()
        warm_mm(40)

        # prologue: everything attention (qb0, h0/h1) needs
        for sb in range(NSB):
            qk_group("q", wqs, 0, sb, sprinkle=1)
        for sb in range(NSB):
            qk_group("k", wks, 0, sb, sprinkle=1)
        for sc in range(4):
            v_group(sc, sprinkle=1)
        # filler consumed during qb0: projections for heads 2-3, V for qb1
        for sb in range(NSB):
            fillers.append(lambda sb=sb: qk_group("q", wqs, 1, sb))
        for sb in range(NSB):
            fillers.append(lambda sb=sb: qk_group("k", wks, 1, sb))
        for sc in range(4, 8):
            fillers.append(lambda sc=sc: v_group(sc))

        # ---- attention: qb-outer so Wo/output-DMA spread across the run ----
        for qb in range(NSB):
            if qb >= 1:
                for sc in range(4 * (qb + 1), 4 * (qb + 2)):
                    if sc < NKC:
                        fillers.append(lambda sc=sc: v_group(sc))
                for sc in range(4 * (qb - 1), 4 * qb):
                    for eb in range(2):
                        fillers.append(
                            lambda sc=sc, eb=eb: wo_group(sc, eb))
            for h in range(HPC):
                if qb == NSB - 1 and h == 2:
                    # h0/h1 of the last q-block are done: their half of the
                    # final output projection can overlap h2/h3
                    for sc in range(12, NKC):
                        for eb in range(2):
                            fillers.append(
                                lambda sc=sc, eb=eb: wo_half(sc, eb))
                cc, po = h // 2, (h % 2) * 64
                qt, kt = qkt["q"][cc], qkt["k"][cc]
                av = avp.tile([65, SBK], F32, tag="av", name=f"av{qb}{h}")
                nk = 4 * (qb + 1)
                pend = deque()  # AV lags scores; emit in same-shape pairs
                def flush_av(nmax, final=False):
                    n = 0
                    while pend and n < nmax:
                        pkc, pj0, ppt = pend.popleft()
                        nc.tensor.matmul(
                            av[:, pj0:SBK],
                            lhsT=vts[pkc][:, h * 65:(h + 1) * 65],
                            rhs=ppt[:, pj0:SBK],
                            start=(pkc == 0),
                            stop=(final and not pend),
                            skip_group_check=True)
                        n += 1
                for kc2 in range(0, nk, 2):
                    for kc in (kc2, kc2 + 1):
                        if kc >= nk:
                            break
                        k0 = kc * 128
                        j0 = max(0, k0 - qb * SBK)
                        ss = sp.tile([128, SBK], F32, tag="sp")
                        nc.tensor.matmul(
                            ss[:, j0:SBK],
                            lhsT=kt[po:po + 64, k0:k0 + 128],
                            rhs=qt[po:po + 64, qb * SBK + j0:(qb + 1) * SBK],
                            start=True, stop=True)
                        pt = ptp.tile([128, SBK], BF16, tag="pt")
                        nc.scalar.activation(pt[:, j0:SBK], ss[:, j0:SBK],
                                             AF.Exp, scale=float(D) ** -0.5)
                        if k0 >= qb * SBK:  # diag chunk: mask 128-wide band
                            nc.vector.tensor_mul(pt[:, j0:j0 + 128],
                                                 pt[:, j0:j0 + 128], tri_sb[:])
                        pend.append((kc, j0, pt))
                        if len(pend) > 2:
                            flush_av(1)
                        if fillers:
                            fillers.popleft()()
                flush_av(99, final=True)
                rec = recp.tile([1, SBK], F32, tag="rec")
                nc.vector.tensor_copy(rec[:], av[64:65, :])
                rec2 = recp.tile([1, SBK], F32, tag="rec2")
                nc.vector.reciprocal_approx_fast(rec2[:], rec[:])
                bc = bcp.tile([64, SBK], F32, tag="bc")
                nc.gpsimd.partition_broadcast(bc[:], rec2[:])
                nc.vector.tensor_mul(
                    ons[cc][po:po + 64, qb * SBK:(qb + 1) * SBK],
                    av[0:64, :], bc[:])

        while fillers:
            fillers.popleft()()
        # tail: combine the remaining half of the last q-block's projection
        for sc in range(12, NKC):
            for eb in range(2):
                wo_combine(sc, eb)


    nc.compile()
    _BUILT["nc"] = nc
    return nc


def _install_ntff_shim():
    """Provide antenv.axon_hooks (missing in this image) so trace=True works."""
    import types
    try:
        from antenv.axon_hooks import get_axon_ntff_profile_hook  # noqa: F401
        return
    except ImportError:
        pass
    import antenv
    from trn_agent_boot.trn_boot import _ntff_profile_via_ctypes
    hook = _ntff_profile_via_ctypes("/opt/axon/libaxon_pjrt.so")
    mod = types.ModuleType("antenv.axon_hooks")
    mod._hook = hook
    mod.get_axon_ntff_profile_hook = lambda: mod._hook
    mod.set_axon_ntff_profile_hook = lambda h: setattr(mod, "_hook", h)
    sys.modules["antenv.axon_hooks"] = mod
    antenv.axon_hooks = mod


def kernel(x, Wq, Wk, Wv, Wo, bo, _trace=False):
    from concourse.bass_utils import run_bass_kernel_spmd

    nc = _build()

    x = np.asarray(x, dtype=np.float32)
    Wq = np.asarray(Wq, dtype=np.float32)
    Wk = np.asarray(Wk, dtype=np.float32)
    Wv = np.asarray(Wv, dtype=np.float32)
    Wo = np.asarray(Wo, dtype=np.float32)
    bo = np.asarray(bo, dtype=np.float32)

    import ml_dtypes
    bf = ml_dtypes.bfloat16
    tri = np.triu(np.ones((128, 128), dtype=np.float32)).astype(bf)
    xt_b = [np.ascontiguousarray(x[b].T) for b in range(B)]
    in_maps = []
    for c in range(N_CORES):
        b, hg = c // HPC, c % HPC
        sl = slice(hg * CH, (hg + 1) * CH)
        in_maps.append({
            "xt": xt_b[b].astype(bf),
            "wq": np.ascontiguousarray(Wq[:, sl]).astype(bf),
            "wk": np.ascontiguousarray(Wk[:, sl]).astype(bf),
            "wv": np.ascontiguousarray(Wv[:, sl]).astype(bf),
            "wo": np.ascontiguousarray(Wo[sl, :]).astype(bf),
            "tri": tri,
        })

    kwargs = {}
    if _trace:
        _install_ntff_shim()
        kwargs = dict(trace=True, trace_cores=[0])
    res = run_bass_kernel_spmd(nc, in_maps, core_ids=list(range(N_CORES)),
                               **kwargs)

    out = np.zeros((B, S, E), dtype=np.float32)
    for c in range(N_CORES):
        out[c // HPC] += res.results[c]["pout"]
    out += bo
    if _trace:
        return out, res
    return out


# revision 27
# speedup vs baseline: 1.0041x; 1.0019x over previous
"""Multi-head causal attention (B=2,S=2048,E=1024,H=16,D=64) on 8 NeuronCores.

Sharding: core c handles batch b=c//4 and head-group hg=c%4 (4 heads = 256
channels each).  Each core computes Q^T/K^T/V projections for its channel
slice, causal softmax attention for its 4 heads, and a partial output
projection through its slice of Wo.  Host sums the 4 partials per batch and
adds the bias.

All matmuls run as float32r (TF32-like: full-rate, ~1.6e-4 rel err).
Attention dataflow is transpose-free:
  scores^T[k,q] = (K^T chunk)^T-stationary @ Q^T      (d on partitions)
  P^T = exp(scores^T * D^-0.5)                        (ACT, causal-restricted)
  out^T[d,q] (+ denom row) = [V|1]^T-stationary @ P^T (k on partitions)
  normalize by broadcasted 1/denom, then
  partial[s,e] = (out_norm^T chunk)-stationary @ Wo-slice
"""

import sys

sys.path.insert(0, "/opt/trn_rl_repo")

import numpy as np

B, S, E, H, D = 2, 2048, 1024, 16, 64
N_CORES = 8
HPC = 4               # heads per core
CH = HPC * D          # 256 channels per core
SBK = 512             # seq block (moving free dim)
NSB = S // SBK        # 4
NE = E // 128         # 8 contraction chunks
NKC = S // 128        # 16 key chunks

_BUILT = {}


def _build():
    if "nc" in _BUILT:
        return _BUILT["nc"]

    import concourse.bass_utils as _bu
    if not getattr(_bu, "_ldw_patched", False):
        _orig_run = _bu.run_command

        def _patched(cmd, *a, **kw):
            cmd = [c.replace("--enable-ldw-opt=false", "--enable-ldw-opt=false")
                   if isinstance(c, str) else c for c in cmd]
            return _orig_run(cmd, *a, **kw)

        _bu.run_command = _patched
        _bu._ldw_patched = True

    from contextlib import ExitStack

    import concourse.bacc as bacc
    import concourse.tile as tile
    from concourse import mybir

    F32 = mybir.dt.float32
    F32R = mybir.dt.float32r
    BF16 = mybir.dt.bfloat16
    AF = mybir.ActivationFunctionType

    nc = bacc.Bacc("TRN2", target_bir_lowering=False, debug=False,
                   num_devices=N_CORES)
    xt = nc.dram_tensor("xt", [E, S], BF16, kind="ExternalInput").ap()
    wq = nc.dram_tensor("wq", [E, CH], BF16, kind="ExternalInput").ap()
    wk = nc.dram_tensor("wk", [E, CH], BF16, kind="ExternalInput").ap()
    wv = nc.dram_tensor("wv", [E, CH], BF16, kind="ExternalInput").ap()
    wo = nc.dram_tensor("wo", [CH, E], BF16, kind="ExternalInput").ap()
    tri = nc.dram_tensor("tri", [128, 128], BF16, kind="ExternalInput").ap()
    pout = nc.dram_tensor("pout", [S, E], F32, kind="ExternalOutput").ap()

    with tile.TileContext(nc) as tc, ExitStack() as ctx:
        wop = ctx.enter_context(tc.tile_pool(name="wop", bufs=2))
        qkp = ctx.enter_context(tc.tile_pool(name="qkp", bufs=4))
        vp = ctx.enter_context(tc.tile_pool(name="vp", bufs=NKC))
        trip = ctx.enter_context(tc.tile_pool(name="trip", bufs=1))
        pp = ctx.enter_context(tc.tile_pool(name="pp", bufs=2, space="PSUM"))
        sp = ctx.enter_context(tc.tile_pool(name="sp", bufs=4, space="PSUM"))
        avp = ctx.enter_context(tc.tile_pool(name="avp", bufs=2, space="PSUM"))
        xtp = ctx.enter_context(tc.tile_pool(name="xtp", bufs=NE))
        wp = ctx.enter_context(tc.tile_pool(name="wp", bufs=3 * NE))

        # --- loads: tri (warmup dep) + x^T first (critical path) ---
        tri_sb = trip.tile([128, 128], BF16, tag="tri")
        nc.sync.dma_start(tri_sb[:], tri[:, :])
        ones_sb = trip.tile([128, HPC], BF16, tag="ones")
        nc.vector.memset(ones_sb[:], 1.0)
        wrm = trip.tile([128, 128], BF16, tag="wrm")
        nc.vector.memset(wrm[:], 0.125)
        xts = []
        for e in range(NE):
            t = xtp.tile([128, S], BF16, tag="xt")
            nc.sync.dma_start(t[:], xt[e * 128:(e + 1) * 128, :])
            xts.append(t)
        wqs, wks, wvs = [], [], []
        for lst, srcw, tg in ((wqs, wq, "wq"), (wks, wk, "wk"),
                              (wvs, wv, "wv")):
            for e in range(NE):
                t = wp.tile([128, CH], BF16, tag=tg)
                nc.gpsimd.dma_start(t[:], srcw[e * 128:(e + 1) * 128, :])
                lst.append(t)
        wos = []
        for cc in range(2):
            t = wop.tile([128, E], BF16, tag="wo")
            nc.gpsimd.dma_start(t[:], wo[cc * 128:(cc + 1) * 128, :])
            wos.append(t)

        onp = ctx.enter_context(tc.tile_pool(name="onp", bufs=2))
        ptp = ctx.enter_context(tc.tile_pool(name="ptp", bufs=8))
        recp = ctx.enter_context(tc.tile_pool(name="recp", bufs=2))
        bcp = ctx.enter_context(tc.tile_pool(name="bcp", bufs=2))
        oop = ctx.enter_context(tc.tile_pool(name="oop", bufs=3))
        ons = [onp.tile([128, S], BF16, tag="on", name=f"on{i}")
               for i in range(2)]

        qkt = {"q": [qkp.tile([128, S], BF16, tag="qk", name=f"qt{i}")
                     for i in range(2)],
               "k": [qkp.tile([128, S], BF16, tag="qk", name=f"kt{i}")
                     for i in range(2)]}
        vts = [vp.tile([128, HPC * 65], BF16, tag="v", name=f"v{i}")
               for i in range(NKC)]

        # ---- dense-matmul group emitters (HAM-warming filler work) ----
        def qk_group(name, wts, cc, sb, sprinkle=0):
            ps = pp.tile([128, SBK], F32, tag="pp", name=f"ps_{name}{cc}{sb}")
            for e in range(NE):
                nc.tensor.matmul(
                    ps[:], lhsT=wts[e][:, cc * 128:(cc + 1) * 128],
                    rhs=xts[e][:, sb * SBK:(sb + 1) * SBK],
                    start=(e == 0), stop=(e == NE - 1))
                if sprinkle:
                    warm_mm(sprinkle)
            nc.scalar.copy(qkt[name][cc][:, sb * SBK:(sb + 1) * SBK], ps[:])

        def v_group(sc, sprinkle=0):
            ps = pp.tile([128, CH], F32, tag="pp", name=f"ps_v{sc}")
            for e in range(NE):
                nc.tensor.matmul(ps[:], lhsT=xts[e][:, sc * 128:(sc + 1) * 128],
                                 rhs=wvs[e][:], start=(e == 0),
                                 stop=(e == NE - 1))
                if sprinkle:
                    warm_mm(sprinkle)
            dst = vts[sc][:].rearrange("p (h c) -> p h c", h=HPC)
            nc.vector.tensor_copy(dst[:, :, 0:64],
                                  ps[:].rearrange("p (h c) -> p h c", h=HPC))
            nc.vector.tensor_copy(dst[:, :, 64:65], ones_sb[:].unsqueeze(2))

        woh = {}

        def wo_half(sc, eb):
            ps = pp.tile([128, SBK], F32, tag="pp", name=f"ph_o{sc}{eb}")
            nc.tensor.matmul(ps[:], lhsT=ons[0][:, sc * 128:(sc + 1) * 128],
                             rhs=wos[0][:, eb * SBK:(eb + 1) * SBK],
                             start=True, stop=True)
            t = oop.tile([128, SBK], F32, tag="oh", bufs=8,
                         name=f"oh{sc}{eb}")
            nc.vector.tensor_copy(t[:], ps[:])
            woh[(sc, eb)] = t

        def wo_combine(sc, eb):
            ps = pp.tile([128, SBK], F32, tag="pp", name=f"pc_o{sc}{eb}")
            nc.tensor.matmul(ps[:], lhsT=ons[1][:, sc * 128:(sc + 1) * 128],
                             rhs=wos[1][:, eb * SBK:(eb + 1) * SBK],
                             start=True, stop=True)
            oo = oop.tile([128, SBK], F32, tag="oo", name=f"oc{sc}{eb}")
            nc.vector.tensor_add(oo[:], ps[:], woh[(sc, eb)][:])
            nc.sync.dma_start(
                pout[sc * 128:(sc + 1) * 128, eb * SBK:(eb + 1) * SBK],
                oo[:])

        def wo_group(sc, eb):
            ps = pp.tile([128, SBK], F32, tag="pp", name=f"ps_o{sc}{eb}")
            for cc in range(2):
                nc.tensor.matmul(ps[:],
                                 lhsT=ons[cc][:, sc * 128:(sc + 1) * 128],
                                 rhs=wos[cc][:, eb * SBK:(eb + 1) * SBK],
                                 start=(cc == 0), stop=(cc == 1))
            oo = oop.tile([128, SBK], F32, tag="oo", name=f"oo{sc}{eb}")
            nc.vector.tensor_copy(oo[:], ps[:])
            nc.sync.dma_start(
                pout[sc * 128:(sc + 1) * 128, eb * SBK:(eb + 1) * SBK],
                oo[:])

        def warm_mm(n):
            for _ in range(n):
                wps = sp.tile([128, 128], F32, tag="sp", name="warmps")
                nc.tensor.matmul(wps[:], lhsT=wrm[:], rhs=wrm[:],
                                 start=True, stop=True)

        from collections import deque
        fillers = deque()
        warm_mm(40)

        # prologue: everything attention (qb0, h0/h1) needs
        for sb in range(NSB):
            qk_group("q", wqs, 0, sb, sprinkle=1)
        for sb in range(NSB):
            qk_group("k", wks, 0, sb, sprinkle=1)
        for sc in range(4):
            v_group(sc)
        # filler consumed during qb0: projections for heads 2-3, V for qb1
        for sb in range(NSB):
            fillers.append(lambda sb=sb: qk_group("q", wqs, 1, sb))
        for sb in range(NSB):
            fillers.append(lambda sb=sb: qk_group("k", wks, 1, sb))
        for sc in range(4, 8):
            fillers.append(lambda sc=sc: v_group(sc))

        # ---- attention: qb-outer so Wo/output-DMA spread across the run ----
        for qb in range(NSB):
            if qb >= 1:
                for sc in range(4 * (qb + 1), 4 * (qb + 2)):
                    if sc < NKC:
                        fillers.append(lambda sc=sc: v_group(sc))
                for sc in range(4 * (qb - 1), 4 * qb):
                    for eb in range(2):
                        fillers.append(
                            lambda sc=sc, eb=eb: wo_group(sc, eb))
            for h in range(HPC):
                if qb == NSB - 1 and h == 2:
                    # h0/h1 of the last q-block are done: their half of the
                    # final output projection can overlap h2/h3
                    for sc in range(12, NKC):
                        for eb in range(2):
                            fillers.append(
                                lambda sc=sc, eb=eb: wo_half(sc, eb))
                cc, po = h // 2, (h % 2) * 64
                qt, kt = qkt["q"][cc], qkt["k"][cc]
                av = avp.tile([65, SBK], F32, tag="av", name=f"av{qb}{h}")
                nk = 4 * (qb + 1)
                pend = deque()  # AV lags scores; emit in same-shape pairs
                def flush_av(nmax, final=False):
                    n = 0
                    while pend and n < nmax:
                        pkc, pj0, ppt = pend.popleft()
                        nc.tensor.matmul(
                            av[:, pj0:SBK],
                            lhsT=vts[pkc][:, h * 65:(h + 1) * 65],
                            rhs=ppt[:, pj0:SBK],
                            start=(pkc == 0),
                            stop=(final and not pend),
                            skip_group_check=True)
                        n += 1
                for kc2 in range(0, nk, 2):
                    for kc in (kc2, kc2 + 1):
                        if kc >= nk:
                            break
                        k0 = kc * 128
                        j0 = max(0, k0 - qb * SBK)
                        ss = sp.tile([128, SBK], F32, tag="sp")
                        nc.tensor.matmul(
                            ss[:, j0:SBK],
                            lhsT=kt[po:po + 64, k0:k0 + 128],
                            rhs=qt[po:po + 64, qb * SBK + j0:(qb + 1) * SBK],
                            start=True, stop=True)
                        pt = ptp.tile([128, SBK], BF16, tag="pt")
                        nc.scalar.activation(pt[:, j0:SBK], ss[:, j0:SBK],
                                             AF.Exp, scale=float(D) ** -0.5)
                        if k0 >= qb * SBK:  # diag chunk: mask 128-wide band
                            nc.vector.tensor_mul(pt[:, j0:j0 + 128],
                                                 pt[:, j0:j0 + 128], tri_sb[:])
                        pend.append((kc, j0, pt))
                        if len(pend) > 2:
                            flush_av(1)
                        if fillers:
                            fillers.popleft()()
                flush_av(99, final=True)
                rec = recp.tile([1, SBK], F32, tag="rec")
                nc.vector.tensor_copy(rec[:], av[64:65, :])
                rec2 = recp.tile([1, SBK], F32, tag="rec2")
                nc.vector.reciprocal_approx_fast(rec2[:], rec[:])
                bc = bcp.tile([64, SBK], F32, tag="bc")
                nc.gpsimd.partition_broadcast(bc[:], rec2[:])
                nc.vector.tensor_mul(
                    ons[cc][po:po + 64, qb * SBK:(qb + 1) * SBK],
                    av[0:64, :], bc[:])

        while fillers:
            fillers.popleft()()
        # tail: combine the remaining half of the last q-block's projection
        for sc in range(12, NKC):
            for eb in range(2):
                wo_combine(sc, eb)


    nc.compile()
    _BUILT["nc"] = nc
    return nc


def _install_ntff_shim():
    """Provide antenv.axon_hooks (missing in this image) so trace=True works."""
    import types
    try:
        from antenv.axon_hooks import get_axon_ntff_profile_hook  # noqa: F401
        return
    except ImportError:
        pass
    import antenv
    from trn_agent_boot.trn_boot import _ntff_profile_via_ctypes
    hook = _ntff_profile_via_ctypes("/opt/axon/libaxon_pjrt.so")
    mod = types.ModuleType("antenv.axon_hooks")
    mod._hook = hook
    mod.get_axon_ntff_profile_hook = lambda: mod._hook
    mod.set_axon_ntff_profile_hook = lambda h: setattr(mod, "_hook", h)
    sys.modules["antenv.axon_hooks"] = mod
    antenv.axon_hooks = mod


def kernel(x, Wq, Wk, Wv, Wo, bo, _trace=False):
    from concourse.bass_utils import run_bass_kernel_spmd

    nc = _build()

    x = np.asarray(x, dtype=np.float32)
    Wq = np.asarray(Wq, dtype=np.float32)
    Wk = np.asarray(Wk, dtype=np.float32)
    Wv = np.asarray(Wv, dtype=np.float32)
    Wo = np.asarray(Wo, dtype=np.float32)
    bo = np.asarray(bo, dtype=np.float32)

    import ml_dtypes
    bf = ml_dtypes.bfloat16
    tri = np.triu(np.ones((128, 128), dtype=np.float32)).astype(bf)
    xt_b = [np.ascontiguousarray(x[b].T) for b in range(B)]
    in_maps = []
    for c in range(N_CORES):
        b, hg = c // HPC, c % HPC
        sl = slice(hg * CH, (hg + 1) * CH)
        in_maps.append({
            "xt": xt_b[b].astype(bf),
            "wq": np.ascontiguousarray(Wq[:, sl]).astype(bf),
            "wk": np.ascontiguousarray(Wk[:, sl]).astype(bf),
            "wv": np.ascontiguousarray(Wv[:, sl]).astype(bf),
            "wo": np.ascontiguousarray(Wo[sl, :]).astype(bf),
            "tri": tri,
        })

    kwargs = {}
    if _trace:
        _install_ntff_shim()
        kwargs = dict(trace=True, trace_cores=[0])
    res = run_bass_kernel_spmd(nc, in_maps, core_ids=list(range(N_CORES)),
                               **kwargs)

    out = np.zeros((B, S, E), dtype=np.float32)
    for c in range(N_CORES):
        out[c // HPC] += res.results[c]["pout"]
    out += bo
    if _trace:
        return out, res
    return out
